# revision 1
# baseline (speedup 1.0000x reference)
"""CombinedCRPSIntervalLoss kernel for 8x TRN2 NeuronCores.

Strategy (pure data parallel over N):
  - shard N across 8 cores; per core, stream noise [S, N/8] through:
      DMA (natural layout) -> PE transpose (128-col blocks) -> ACT Exp
      (samples = exp(mu + sig_c * z), written bf16, [col->partition,
      S->free] layout) -> DVE bitonic sort (128-wide, 28 stages, zero
      padded: 100 real + 28 zero pads sort to front) -> GPSIMD
      coefficient-weighted sum (exact sorted-order CRPS identity)
      + ACT Abs pass for term1, accum on device.
  - interval score phase is tiny elementwise work on [N/8] vectors.
  - each core emits 18 fp32 partial-sum columns; host combines in fp64.

Math identity used (validated vs reference to ~1e-7 rel):
  sum_{i,j}|s_i - s_j| = 2 * sum_k (2k+1-S) s_(k)   (ascending sort)
  With 28 zero pads sorted to the front of 128 slots, coefficient at
  slot k becomes (2k - 155); pads contribute exactly 0.
"""

import os
import sys
import numpy as np

S = 100
N_TOTAL = 500000
NCORES = 8
N_LOC = N_TOTAL // NCORES          # 62500
C_FULL = 32                        # column-slots per partition per sort tile
EPS = 1e-6
ALPHA = 0.1
Z_LO = -1.6448536269514729         # norm.ppf(0.05)
Z_HI = 1.6448536269514722          # norm.ppf(0.95)
PEN_W = 2.0 / ALPHA                # 20.0

_STATE = {}


def _install_axon_hook_shim():
    """bass_utils imports antenv.axon_hooks when trace=True under axon;
    this image's antenv lacks it. Register a lazy shim so tracing works
    (and trace=False paths are unaffected)."""
    import types
    try:
        import antenv.axon_hooks  # noqa: F401
        return
    except ImportError:
        pass
    mod = types.ModuleType("antenv.axon_hooks")
    _state = {"hook": None, "built": False}

    def set_axon_ntff_profile_hook(h):
        _state["hook"] = h
        _state["built"] = True

    def get_axon_ntff_profile_hook():
        if not _state["built"]:
            _state["built"] = True
            try:
                from trn_agent_boot.trn_boot import _ntff_profile_via_ctypes
                _state["hook"] = _ntff_profile_via_ctypes("/opt/axon/libaxon_pjrt.so")
            except Exception:
                _state["hook"] = None
        return _state["hook"]

    mod.set_axon_ntff_profile_hook = set_axon_ntff_profile_hook
    mod.get_axon_ntff_profile_hook = get_axon_ntff_profile_hook
    sys.modules["antenv.axon_hooks"] = mod
    try:
        import antenv
        antenv.axon_hooks = mod
    except Exception:
        pass


def _split_drain_waits(nc):
    """This walrus build allows only one sem wait per TPB instruction on
    several engine paths (CTRL drain, Pool STT); hoist extra waits onto
    EventSemaphore instructions inserted before (same engine => same
    semantics)."""
    import concourse.mybir as mybir
    for f in nc.m.functions:
        for b in f.blocks:
            new_insts = []
            for inst in b.instructions:
                si = inst.sync_info
                if (not isinstance(inst, mybir.InstEventSemaphore)
                        and si is not None
                        and si.on_wait and len(si.on_wait) > 1):
                    waits = list(si.on_wait)
                    for i, w in enumerate(waits[:-1]):
                        new_insts.append(mybir.InstEventSemaphore(
                            name=f"{inst.name}-dw{i}",
                            engine=inst.engine,
                            ins=[], outs=[],
                            sync_info=mybir.SyncInfo(on_wait=[w], on_update=[]),
                        ))
                    si.on_wait = [waits[-1]]
                new_insts.append(inst)
            b.instructions = new_insts


def _tiles_for(n_pad):
    """Split n_pad columns (multiple of 128) into sort tiles of <=32
    column-slots per partition."""
    slots = n_pad // 128
    tiles = []
    f = 0
    while f < slots:
        c = min(C_FULL, slots - f)
        tiles.append((f, c))
        f += c
    return tiles


def _emit_sort(nc, bass, mybir, A, B, C):
    """Bitonic sort ascending along the innermost 128 of A[:, 0:C, :]
    (bf16), ping-pong via B. 28 stages; result lands back in A."""
    amin = mybir.AluOpType.min
    amax = mybir.AluOpType.max

    def rev_tail(V, lo, cnt):
        step = V.ap[-1][0]
        return bass.AP(tensor=V.tensor, offset=V.offset + (lo + cnt - 1) * step,
                       ap=[*V.ap[:-1], [-step, cnt]])

    cur, nxt = A, B
    nstages = 0
    for m in (2, 4, 8, 16, 32, 64, 128):
        nb = 128 // m
        h = m // 2
        Vc = cur[:, 0:C, :].rearrange("p c (nb m) -> p c nb m", m=m)
        Vn = nxt[:, 0:C, :].rearrange("p c (nb m) -> p c nb m", m=m)
        lo_in = Vc[:, :, :, 0:h]
        hi_in = rev_tail(Vc, h, h)
        nc.vector.tensor_tensor(out=Vn[:, :, :, 0:h], in0=lo_in, in1=hi_in, op=amin)
        nc.vector.tensor_tensor(out=rev_tail(Vn, h, h), in0=lo_in, in1=hi_in, op=amax)
        cur, nxt = nxt, cur
        nstages += 1
        d = m // 4
        while d >= 1:
            Wc = cur[:, 0:C, :].rearrange("p c (nb two d) -> p c nb two d", two=2, d=d)
            Wn = nxt[:, 0:C, :].rearrange("p c (nb two d) -> p c nb two d", two=2, d=d)
            a_in = Wc[:, :, :, 0, :]
            b_in = Wc[:, :, :, 1, :]
            nc.vector.tensor_tensor(out=Wn[:, :, :, 0, :], in0=a_in, in1=b_in, op=amin)
            nc.vector.tensor_tensor(out=Wn[:, :, :, 1, :], in0=a_in, in1=b_in, op=amax)
            cur, nxt = nxt, cur
            nstages += 1
            d //= 2
    assert nstages == 28 and cur is A, (nstages, cur is A)


def _build(n_pad):
    """Build the per-core Bass module for n_pad padded columns."""
    import concourse.bass as bass
    import concourse.mybir as mybir
    import concourse.tile as tile

    f32 = mybir.dt.float32
    bf16 = mybir.dt.bfloat16
    slots = n_pad // 128
    tiles = _tiles_for(n_pad)
    ntiles = len(tiles)
    nout = ntiles + 2  # weighted sums per tile, term1 total, interval total

    nc = bass.Bass("TRN2", target_bir_lowering=False, debug=False, num_devices=1)

    noise_d = nc.dram_tensor("noise", [S, n_pad], f32, kind="ExternalInput")
    mu_d = nc.dram_tensor("mu_t", [128, slots], f32, kind="ExternalInput")
    sig_d = nc.dram_tensor("sig_t", [128, slots], f32, kind="ExternalInput")
    sigc_d = nc.dram_tensor("sigc_t", [128, slots], f32, kind="ExternalInput")
    tgt_d = nc.dram_tensor("tgt_t", [128, slots], f32, kind="ExternalInput")
    ntgtc_d = nc.dram_tensor("ntgtc_t", [128, slots], f32, kind="ExternalInput")
    coef_d = nc.dram_tensor("coef", [128, C_FULL * 128], bf16, kind="ExternalInput")
    ident_d = nc.dram_tensor("ident", [128, 128], f32, kind="ExternalInput")
    part_d = nc.dram_tensor("partials", [128, nout], f32, kind="ExternalOutput")

    aE = mybir.ActivationFunctionType.Exp
    aA = mybir.ActivationFunctionType.Abs
    X = mybir.AxisListType.X
    op_add = mybir.AluOpType.add
    op_sub = mybir.AluOpType.subtract
    op_mul = mybir.AluOpType.mult
    op_lt = mybir.AluOpType.is_lt
    op_gt = mybir.AluOpType.is_gt

    with tile.TileContext(nc) as tc:
        with (
            tc.tile_pool(name="singles", bufs=1) as singles,
            tc.tile_pool(name="nzp", bufs=2) as nzp,
            tc.tile_pool(name="sortp", bufs=2) as sortp,
            tc.tile_pool(name="wsp", bufs=2) as wsp,
            tc.tile_pool(name="psump", bufs=4, space="PSUM") as psump,
        ):
            # --- load per-column constants & helpers ---
            mu_s = singles.tile([128, slots], f32, tag="mu_s")
            sig_s = singles.tile([128, slots], f32, tag="sig_s")
            sigc_s = singles.tile([128, slots], f32, tag="sigc_s")
            tgt_s = singles.tile([128, slots], f32, tag="tgt_s")
            ntgtc_s = singles.tile([128, slots], f32, tag="ntgtc_s")
            coef_s = singles.tile([128, C_FULL * 128], bf16, tag="coef_s")
            ident_s = singles.tile([128, 128], f32, tag="ident_s")
            for sb, dr in ((mu_s, mu_d), (sig_s, sig_d), (sigc_s, sigc_d),
                           (tgt_s, tgt_d), (ntgtc_s, ntgtc_d), (coef_s, coef_d),
                           (ident_s, ident_d)):
                nc.sync.dma_start(out=sb[:, :], in_=dr.ap())

            t1buf = singles.tile([128, slots], f32, tag="t1buf")
            outbuf = singles.tile([128, nout], f32, tag="outbuf")

            # --- main streaming loop over sort tiles ---
            for ti, (f0, C) in enumerate(tiles):
                nz = nzp.tile([S, C_FULL * 128], f32, tag="nz")
                nc.sync.dma_start(
                    out=nz[0:S, 0:C * 128],
                    in_=noise_d.ap()[0:S, f0 * 128:(f0 + C) * 128],
                )
                A = sortp.tile([128, C_FULL, 128], bf16, tag="A")
                B = sortp.tile([128, C_FULL, 128], bf16, tag="B")
                nc.scalar.memzero(A[:, 0:C, S:128])
                for c in range(C):
                    f = f0 + c
                    pt = psump.tile([128, S], f32, tag="pt")
                    nc.tensor.transpose(
                        pt[:, :], nz[0:S, c * 128:(c + 1) * 128],
                        ident_s[0:S, 0:S],
                    )
                    nc.scalar.activation(
                        A[:, c, 0:S], pt[:, :], aE,
                        bias=mu_s[:, f:f + 1], scale=sigc_s[:, f:f + 1],
                    )
                    # |s - t_c|, summed over the free axis into t1buf[:, f]
                    nc.scalar.activation(
                        B[:, c, 0:S], A[:, c, 0:S], aA,
                        bias=ntgtc_s[:, f:f + 1],
                        accum_out=t1buf[:, f:f + 1],
                    )
                _emit_sort(nc, bass, mybir, A, B, C)
                # weighted sum: sum_k coef_k * sorted_k  (pads hit coef*0)
                wscr = wsp.tile([128, C_FULL * 128], bf16, tag="wscr")
                Aflat = A[:, 0:C, :].rearrange("p c k -> p (c k)")
                nc.vector.tensor_tensor(
                    out=wscr[:, 0:C * 128], in0=Aflat,
                    in1=coef_s[:, 0:C * 128], op=op_mul)
                nc.vector.tensor_reduce(
                    out=outbuf[:, ti:ti + 1], in_=wscr[:, 0:C * 128],
                    axis=X, op=op_add)

            # --- interval score phase (elementwise over [128, slots]) ---
            iv = [singles.tile([128, slots], f32, tag=f"iv{i}", name=f"iv{i}") for i in range(7)]
            lo_a, hi_a, low, upp, bel, abv, pen = iv
            nc.vector.scalar_tensor_tensor(
                out=lo_a[:, :], in0=sig_s[:, :], scalar=Z_LO, in1=mu_s[:, :],
                op0=op_mul, op1=op_add)
            nc.vector.scalar_tensor_tensor(
                out=hi_a[:, :], in0=sig_s[:, :], scalar=Z_HI, in1=mu_s[:, :],
                op0=op_mul, op1=op_add)
            nc.scalar.activation(low[:, :], lo_a[:, :], aE)
            nc.scalar.activation(upp[:, :], hi_a[:, :], aE)
            nc.vector.tensor_tensor(out=bel[:, :], in0=tgt_s[:, :], in1=low[:, :], op=op_lt)
            nc.vector.tensor_tensor(out=abv[:, :], in0=tgt_s[:, :], in1=upp[:, :], op=op_gt)
            # reuse lo_a/hi_a as diff scratch
            nc.vector.tensor_tensor(out=lo_a[:, :], in0=low[:, :], in1=tgt_s[:, :], op=op_sub)
            nc.vector.tensor_tensor(out=hi_a[:, :], in0=tgt_s[:, :], in1=upp[:, :], op=op_sub)
            nc.vector.tensor_tensor(out=bel[:, :], in0=lo_a[:, :], in1=bel[:, :], op=op_mul)
            nc.vector.tensor_tensor(out=abv[:, :], in0=hi_a[:, :], in1=abv[:, :], op=op_mul)
            nc.vector.tensor_tensor(out=pen[:, :], in0=bel[:, :], in1=abv[:, :], op=op_add)
            nc.vector.tensor_tensor(out=upp[:, :], in0=upp[:, :], in1=low[:, :], op=op_sub)
            nc.vector.scalar_tensor_tensor(
                out=low[:, :], in0=pen[:, :], scalar=PEN_W, in1=upp[:, :],
                op0=op_mul, op1=op_add,
                accum_out=outbuf[:, ntiles + 1:ntiles + 2])

            # --- finalize: term1 total and DMA out ---
            nc.vector.tensor_reduce(
                out=outbuf[:, ntiles:ntiles + 1], in_=t1buf[:, :], axis=X, op=op_add)
            nc.sync.dma_start(out=part_d.ap(), in_=outbuf[:, :])

    _split_drain_waits(nc)
    return nc, ntiles, nout


def _get_built(n_pad):
    key = ("nc", n_pad)
    if key not in _STATE:
        _install_axon_hook_shim()
        _STATE[key] = _build(n_pad)
    return _STATE[key]


def _prep_core_inputs(mu, sigma, target, noise, lo, hi, n_pad):
    import ml_dtypes
    n = hi - lo
    slots = n_pad // 128

    def pad_t(vec, fill):
        p = np.full(n_pad, fill, np.float32)
        p[:n] = vec[lo:hi]
        return np.ascontiguousarray(p.reshape(slots, 128).T)

    mu_t = pad_t(mu, 0.0)
    sig_t = pad_t(sigma, 0.0)
    sigc_t = np.maximum(sig_t, EPS)
    tgt_t = pad_t(target, 1.0)
    ntgtc_t = -np.maximum(tgt_t, EPS)

    noise_p = np.zeros((S, n_pad), np.float32)
    noise_p[:, :n] = noise[:, lo:hi]

    coef = (2.0 * np.arange(128, dtype=np.float32) - 155.0)
    coef_w = np.broadcast_to(np.tile(coef, C_FULL), (128, C_FULL * 128))
    coef_w = np.ascontiguousarray(coef_w).astype(ml_dtypes.bfloat16)

    return {
        "noise": noise_p,
        "mu_t": mu_t, "sig_t": sig_t, "sigc_t": sigc_t,
        "tgt_t": tgt_t, "ntgtc_t": ntgtc_t,
        "coef": coef_w,
        "ident": np.eye(128, dtype=np.float32),
    }


def _run(mu, sigma, target, noise, n_loc=N_LOC, ncores=NCORES):
    from concourse import bass_utils

    n_pad = ((n_loc + 4095) // 4096) * 4096
    if n_pad - n_loc >= 4096 - 1152 and (n_loc % 128) <= 1152:
        # shrink tail tile instead of a full pad tile
        n_pad = (n_loc // 4096) * 4096 + max(1152, ((n_loc % 4096 + 127) // 128) * 128)
    n_pad = max(n_pad, 4096)
    nc, ntiles, nout = _get_built(n_pad)

    in_maps = []
    for c in range(ncores):
        in_maps.append(_prep_core_inputs(
            mu, sigma, target, noise, c * n_loc, (c + 1) * n_loc, n_pad))

    res = bass_utils.run_bass_kernel_spmd(
        nc, in_maps, core_ids=list(range(ncores)))
    _STATE["last_result"] = res

    t1 = w = iv = 0.0
    for c in range(ncores):
        p = res.results[c]["partials"].astype(np.float64)
        w += p[:, 0:ntiles].sum()
        t1 += p[:, ntiles].sum()
        iv += p[:, ntiles + 1].sum()
    n_total = n_loc * ncores
    loss = (t1 / S - w / (S * S) + iv) / n_total
    return np.float32(loss)


def kernel(mu, sigma, target, noise):
    mu = np.asarray(mu, dtype=np.float32)
    sigma = np.asarray(sigma, dtype=np.float32)
    target = np.asarray(target, dtype=np.float32)
    noise = np.asarray(noise, dtype=np.float32)
    return _run(mu, sigma, target, noise)



# revision 10
# speedup vs baseline: 1.4157x; 1.4157x over previous
"""CombinedCRPSIntervalLoss kernel for 8x TRN2 NeuronCores (v2).

Strategy (pure data parallel over N):
  - shard N across 8 cores; host pre-transposes noise to per-core
    [128 part][slot][k] bf16 so each sort tile DMAs as one contiguous
    line per partition (no PE transpose / PSUM needed).
  - per (tile, column): ACT computes s = exp(mu + sig_c * z) straight
    into a [p, k, c] layout (c contiguous innermost), then |s - t_c|
    with fused accumulation for term1.
  - order statistics via a Batcher merge-exchange network for exactly
    n=100 (Knuth 5.2.2M; 1077 comparators vs 1792 for bitonic-128),
    vectorized with the sort axis strided and columns contiguous so
    every DVE min/max runs in the packed 2x bf16 mode. Ping-pong
    buffer residency is tracked per position at schedule-build time
    (no copy passes); final weighted sum (exact sorted-order CRPS
    identity) reads each residency segment via fused
    tensor_tensor_reduce ops chained through their accumulator.
    The last (short) tile sorts on GpSimd to overlap with DVE.
  - interval score phase is tiny elementwise work on [128, slots].
  - each core emits fp32 partial-sum columns; host combines in fp64.

Math identity used (validated vs reference):
  sum_{i,j}|s_i - s_j| = 2 * sum_k (2k + 1 - S) * s_(k)  (ascending)
"""

import math
import sys

import numpy as np

S = 100
N_TOTAL = 500000
NCORES = 8
N_LOC = N_TOTAL // NCORES          # 62500
SLOTS = 490                        # 490*128 = 62720 >= 62500
N_PAD = SLOTS * 128
TILE_CS = (64, 64, 64, 64, 64, 64, 64, 42)
GP_TILES = ()                      # Pool has no ALU tensor ops on TRN2
EPS = 1e-6
ALPHA = 0.1
Z_LO = -1.6448536269514729         # norm.ppf(0.05)
Z_HI = 1.6448536269514722          # norm.ppf(0.95)
PEN_W = 2.0 / ALPHA                # 20.0

_STATE = {}


# ---------------------------------------------------------------------------
# sort schedule: Batcher merge-exchange for n=S with ping-pong residency
# ---------------------------------------------------------------------------

def _merge_exchange_classes(n):
    t = math.ceil(math.log2(n))
    classes = []
    p = 2 ** (t - 1)
    while p >= 1:
        q = 2 ** (t - 1)
        r = 0
        d = p
        while True:
            ilist = [i for i in range(n - d) if (i & p) == r]
            if ilist:
                classes.append((d, ilist))
            if q == p:
                break
            d = q - p
            q //= 2
            r = p
        p //= 2
    return classes


def _decompose_2level(lst):
    """Cover a sorted index list by blocks {a + u*P + v : u<reps, v<bl}."""
    remaining = set(lst)
    blocks = []
    while remaining:
        start = min(remaining)
        bl = 1
        while start + bl in remaining:
            bl += 1
        best = (start, 1, 1, bl)
        best_cov = bl
        for P in (2, 4, 8, 16, 32, 64, 128):
            if P < bl:
                continue
            reps = 1
            while all(start + reps * P + v in remaining for v in range(bl)):
                reps += 1
            cov = reps * bl
            if cov > best_cov:
                best_cov = cov
                best = (start, P, reps, bl)
        blocks.append(best)
        st, P, reps, bl = best
        for u in range(reps):
            for v in range(bl):
                remaining.discard(st + u * P + v)
    return blocks


def _build_sort_schedule(n):
    """Returns (sched, final_segments):
    sched: list of (d, (start, period, reps, blocklen), res_lo, res_hi)
      outputs go to buffers 1-res_lo / 1-res_hi (clobber-free: within a
      class the i and i+d position sets are disjoint since d = p mod 2p).
    final_segments: list of (k0, length, residency) covering [0, n)."""
    res = [0] * n
    sched = []
    for d, il in _merge_exchange_classes(n):
        groups = {}
        for i in il:
            groups.setdefault((res[i], res[i + d]), []).append(i)
        for (rlo, rhi), lst in sorted(groups.items()):
            for blk in _decompose_2level(lst):
                sched.append((d, blk, rlo, rhi))
        for i in il:
            res[i] = 1 - res[i]
            res[i + d] = 1 - res[i + d]
    segs = []
    start = 0
    for k in range(1, n + 1):
        if k == n or res[k] != res[start]:
            segs.append((start, k - start, res[start]))
            start = k
    return sched, segs


_SORT_SCHED, _FINAL_SEGS = _build_sort_schedule(S)


def _selfcheck_schedule():
    rng = np.random.default_rng(0)
    for _ in range(20):
        x = rng.standard_normal(S)
        bufs = [x.copy(), np.zeros(S)]
        for d, (st, P, reps, bl), rlo, rhi in _SORT_SCHED:
            idx = (st + P * np.arange(reps)[:, None]
                   + np.arange(bl)[None, :]).ravel()
            a = bufs[rlo][idx].copy()
            b = bufs[rhi][idx + d].copy()
            bufs[1 - rlo][idx] = np.minimum(a, b)
            bufs[1 - rhi][idx + d] = np.maximum(a, b)
        out = np.empty(S)
        for k0, ln, r in _FINAL_SEGS:
            out[k0:k0 + ln] = bufs[r][k0:k0 + ln]
        assert np.array_equal(out, np.sort(x))


_selfcheck_schedule()


# ---------------------------------------------------------------------------
# axon shim + sync-wait splitting (unchanged from v1)
# ---------------------------------------------------------------------------

def _install_axon_hook_shim():
    """bass_utils imports antenv.axon_hooks when trace=True under axon;
    this image's antenv lacks it. Register a lazy shim so tracing works
    (and trace=False paths are unaffected)."""
    import types
    try:
        import antenv.axon_hooks  # noqa: F401
        return
    except ImportError:
        pass
    mod = types.ModuleType("antenv.axon_hooks")
    _state = {"hook": None, "built": False}

    def set_axon_ntff_profile_hook(h):
        _state["hook"] = h
        _state["built"] = True

    def get_axon_ntff_profile_hook():
        if not _state["built"]:
            _state["built"] = True
            try:
                from trn_agent_boot.trn_boot import _ntff_profile_via_ctypes
                _state["hook"] = _ntff_profile_via_ctypes("/opt/axon/libaxon_pjrt.so")
            except Exception:
                _state["hook"] = None
        return _state["hook"]

    mod.set_axon_ntff_profile_hook = set_axon_ntff_profile_hook
    mod.get_axon_ntff_profile_hook = get_axon_ntff_profile_hook
    sys.modules["antenv.axon_hooks"] = mod
    try:
        import antenv
        antenv.axon_hooks = mod
    except Exception:
        pass


def _split_drain_waits(nc):
    """This walrus build allows only one sem wait per TPB instruction on
    several engine paths (CTRL drain, Pool STT); hoist extra waits onto
    EventSemaphore instructions inserted before (same engine => same
    semantics)."""
    import concourse.mybir as mybir
    for f in nc.m.functions:
        for b in f.blocks:
            new_insts = []
            for inst in b.instructions:
                si = inst.sync_info
                if (not isinstance(inst, mybir.InstEventSemaphore)
                        and si is not None
                        and si.on_wait and len(si.on_wait) > 1):
                    waits = list(si.on_wait)
                    for i, w in enumerate(waits[:-1]):
                        new_insts.append(mybir.InstEventSemaphore(
                            name=f"{inst.name}-dw{i}",
                            engine=inst.engine,
                            ins=[], outs=[],
                            sync_info=mybir.SyncInfo(on_wait=[w], on_update=[]),
                        ))
                    si.on_wait = [waits[-1]]
                new_insts.append(inst)
            b.instructions = new_insts


# ---------------------------------------------------------------------------
# device kernel
# ---------------------------------------------------------------------------

def _mk_ap(bass, buf, koff, P, reps, bl, C):
    """AP over the [128, S, Cdim] tile `buf` selecting k in
    {koff + u*P + v : u<reps, v<bl} x c in [0, C)."""
    v = buf[:, koff:koff + 1, 0:C]
    kstep = v.ap[1][0]
    ap = [v.ap[0]]
    if reps > 1:
        ap.append([P * kstep, reps])
    ap.append([kstep, bl])
    ap.append([1, C])
    return bass.AP(tensor=v.tensor, offset=v.offset, ap=ap)


def _build():
    import concourse.bass as bass
    import concourse.mybir as mybir
    import concourse.tile as tile

    f32 = mybir.dt.float32
    bf16 = mybir.dt.bfloat16
    ntiles = len(TILE_CS)
    nout = ntiles + 2  # per-tile wsum | t1 total | interval

    nc = bass.Bass("TRN2", target_bir_lowering=False, debug=False,
                   num_devices=1)

    noise_d = nc.dram_tensor("noise", [128, SLOTS * S], bf16,
                             kind="ExternalInput")
    mu_d = nc.dram_tensor("mu_t", [128, SLOTS], f32, kind="ExternalInput")
    sig_d = nc.dram_tensor("sig_t", [128, SLOTS], f32, kind="ExternalInput")
    sigc_d = nc.dram_tensor("sigc_t", [128, SLOTS], f32, kind="ExternalInput")
    tgt_d = nc.dram_tensor("tgt_t", [128, SLOTS], f32, kind="ExternalInput")
    ntgtc_d = nc.dram_tensor("ntgtc_t", [128, SLOTS], f32,
                             kind="ExternalInput")
    coef_d = nc.dram_tensor("coef", [128, S * 64], bf16, kind="ExternalInput")
    part_d = nc.dram_tensor("partials", [128, nout], f32,
                            kind="ExternalOutput")

    aE = mybir.ActivationFunctionType.Exp
    aA = mybir.ActivationFunctionType.Abs
    X = mybir.AxisListType.X
    op_add = mybir.AluOpType.add
    op_sub = mybir.AluOpType.subtract
    op_mul = mybir.AluOpType.mult
    op_min = mybir.AluOpType.min
    op_byp = mybir.AluOpType.bypass
    op_max = mybir.AluOpType.max
    op_lt = mybir.AluOpType.is_lt
    op_gt = mybir.AluOpType.is_gt

    with tile.TileContext(nc) as tc:
        with (
            tc.tile_pool(name="singles", bufs=1) as singles,
            tc.tile_pool(name="nzp", bufs=2) as nzp,
            tc.tile_pool(name="sortp", bufs=2) as sortp,
            tc.tile_pool(name="gpool", bufs=1) as gpool,
        ):
            mu_s = singles.tile([128, SLOTS], f32, tag="mu_s")
            sig_s = singles.tile([128, SLOTS], f32, tag="sig_s")
            sigc_s = singles.tile([128, SLOTS], f32, tag="sigc_s")
            tgt_s = singles.tile([128, SLOTS], f32, tag="tgt_s")
            ntgtc_s = singles.tile([128, SLOTS], f32, tag="ntgtc_s")
            coef_s = singles.tile([128, S * 64], bf16, tag="coef_s")
            for sb, dr in ((mu_s, mu_d), (sig_s, sig_d), (sigc_s, sigc_d),
                           (tgt_s, tgt_d), (ntgtc_s, ntgtc_d),
                           (coef_s, coef_d)):
                nc.sync.dma_start(out=sb[:, :], in_=dr.ap())

            t1buf = singles.tile([128, SLOTS], f32, tag="t1buf")
            outbuf = singles.tile([128, nout], f32, tag="outbuf")
            wscr = singles.tile([128, S * 64], bf16, tag="wscr")

            slot_of = []
            s0 = 0
            for C in TILE_CS:
                slot_of.append(s0)
                s0 += C

            def emit_compute(ti, eng_tag):
                """DMA + ACT exp/abs + sort for tile ti on the given engine.
                Returns the (A, B) tile pair for the later weighted sum."""
                C = TILE_CS[ti]
                slot0 = slot_of[ti]
                eng = nc.gpsimd if eng_tag == "g" else nc.vector
                if eng_tag == "g":
                    nz = gpool.tile([128, C * S], bf16, tag=f"nzg{ti}")
                    A = gpool.tile([128, S, C], bf16, tag=f"Ag{ti}")
                    B = gpool.tile([128, S, C], bf16, tag=f"Bg{ti}")
                else:
                    nz = nzp.tile([128, 64 * S], bf16, tag="nz")
                    A = sortp.tile([128, S, 64], bf16, tag="A")
                    B = sortp.tile([128, S, 64], bf16, tag="B")
                nc.sync.dma_start(
                    out=nz[:, 0:C * S],
                    in_=noise_d.ap()[:, slot0 * S:(slot0 + C) * S],
                )
                # s = exp(mu + sigc * z), one ACT op per 128-column block
                for c in range(C):
                    f = slot0 + c
                    nc.scalar.activation(
                        A[:, 0:S, c], nz[:, c * S:(c + 1) * S], aE,
                        bias=mu_s[:, f:f + 1], scale=sigc_s[:, f:f + 1],
                    )
                # |s - t_c| with fused column accumulation for term1
                for c in range(C):
                    f = slot0 + c
                    nc.scalar.activation(
                        B[:, 0:S, c], A[:, 0:S, c], aA,
                        bias=ntgtc_s[:, f:f + 1],
                        accum_out=t1buf[:, f:f + 1],
                    )
                # merge-exchange sort over k (c contiguous => packed mode).
                # Pool has no TensorTensor opcode; use STT with op0=bypass.
                bufs = (A, B)
                for d, (st, P, reps, bl), rlo, rhi in _SORT_SCHED:
                    a_in = _mk_ap(bass, bufs[rlo], st, P, reps, bl, C)
                    b_in = _mk_ap(bass, bufs[rhi], st + d, P, reps, bl, C)
                    a_out = _mk_ap(bass, bufs[1 - rlo], st, P, reps, bl, C)
                    b_out = _mk_ap(bass, bufs[1 - rhi], st + d, P, reps, bl, C)
                    if eng_tag == "g":
                        eng.scalar_tensor_tensor(
                            out=a_out, in0=a_in, scalar=0.0, in1=b_in,
                            op0=op_byp, op1=op_min)
                        eng.scalar_tensor_tensor(
                            out=b_out, in0=a_in, scalar=0.0, in1=b_in,
                            op0=op_byp, op1=op_max)
                    else:
                        eng.tensor_tensor(out=a_out, in0=a_in, in1=b_in,
                                          op=op_min)
                        eng.tensor_tensor(out=b_out, in0=a_in, in1=b_in,
                                          op=op_max)
                return A, B

            def emit_wsum(ti, A, B):
                """Weighted sum over sorted values (DVE): per-segment
                coef*sorted into wscr, then one fused reduce."""
                C = TILE_CS[ti]
                bufs = (A, B)
                for k0, ln, r in _FINAL_SEGS:
                    in0 = _mk_ap(bass, bufs[r], k0, 1, 1, ln, C)
                    cv = coef_s[:, k0 * 64:k0 * 64 + 1]
                    in1 = bass.AP(tensor=cv.tensor, offset=cv.offset,
                                  ap=[cv.ap[0], [64, ln], [1, C]])
                    wv = wscr[:, k0 * 64:k0 * 64 + 1]
                    out = bass.AP(tensor=wv.tensor, offset=wv.offset,
                                  ap=[wv.ap[0], [64, ln], [1, C]])
                    nc.vector.tensor_tensor(out=out, in0=in0, in1=in1,
                                            op=op_mul)
                wfull = wscr[:, 0:1]
                win = bass.AP(tensor=wfull.tensor, offset=wfull.offset,
                              ap=[wfull.ap[0], [64, S], [1, C]])
                nc.vector.tensor_reduce(
                    out=outbuf[:, ti:ti + 1], in_=win,
                    axis=mybir.AxisListType.XY, op=op_add)

            # GpSimd tile first so it starts sorting ASAP
            gp_bufs = {}
            for ti in GP_TILES:
                gp_bufs[ti] = emit_compute(ti, "g")
            dve_tiles = [ti for ti in range(ntiles) if ti not in GP_TILES]
            for ti in dve_tiles:
                A, B = emit_compute(ti, "v")
                emit_wsum(ti, A, B)
            for ti in GP_TILES:
                emit_wsum(ti, *gp_bufs[ti])

            # --- interval score phase (elementwise over [128, SLOTS]) ---
            iv = [singles.tile([128, SLOTS], f32, tag=f"iv{i}",
                               name=f"iv{i}") for i in range(7)]
            lo_a, hi_a, low, upp, bel, abv, pen = iv
            nc.vector.scalar_tensor_tensor(
                out=lo_a[:, :], in0=sig_s[:, :], scalar=Z_LO, in1=mu_s[:, :],
                op0=op_mul, op1=op_add)
            nc.vector.scalar_tensor_tensor(
                out=hi_a[:, :], in0=sig_s[:, :], scalar=Z_HI, in1=mu_s[:, :],
                op0=op_mul, op1=op_add)
            nc.scalar.activation(low[:, :], lo_a[:, :], aE)
            nc.scalar.activation(upp[:, :], hi_a[:, :], aE)
            nc.vector.tensor_tensor(out=bel[:, :], in0=tgt_s[:, :],
                                    in1=low[:, :], op=op_lt)
            nc.vector.tensor_tensor(out=abv[:, :], in0=tgt_s[:, :],
                                    in1=upp[:, :], op=op_gt)
            nc.vector.tensor_tensor(out=lo_a[:, :], in0=low[:, :],
                                    in1=tgt_s[:, :], op=op_sub)
            nc.vector.tensor_tensor(out=hi_a[:, :], in0=tgt_s[:, :],
                                    in1=upp[:, :], op=op_sub)
            nc.vector.tensor_tensor(out=bel[:, :], in0=lo_a[:, :],
                                    in1=bel[:, :], op=op_mul)
            nc.vector.tensor_tensor(out=abv[:, :], in0=hi_a[:, :],
                                    in1=abv[:, :], op=op_mul)
            nc.vector.tensor_tensor(out=pen[:, :], in0=bel[:, :],
                                    in1=abv[:, :], op=op_add)
            nc.vector.tensor_tensor(out=upp[:, :], in0=upp[:, :],
                                    in1=low[:, :], op=op_sub)
            nc.vector.scalar_tensor_tensor(
                out=low[:, :], in0=pen[:, :], scalar=PEN_W, in1=upp[:, :],
                op0=op_mul, op1=op_add,
                accum_out=outbuf[:, ntiles + 1:ntiles + 2])

            # term1 total
            nc.vector.tensor_reduce(
                out=outbuf[:, ntiles:ntiles + 1], in_=t1buf[:, :],
                axis=X, op=op_add)
            nc.sync.dma_start(out=part_d.ap(), in_=outbuf[:, :])

    _split_drain_waits(nc)
    return nc, ntiles, nout


def _get_built():
    if "nc" not in _STATE:
        _install_axon_hook_shim()
        _STATE["nc"] = _build()
    return _STATE["nc"]


# ---------------------------------------------------------------------------
# host-side data marshaling
# ---------------------------------------------------------------------------

def _prep_core_inputs(mu, sigma, target, noise_bt, lo, hi, coef_w):
    n = hi - lo

    def pad_t(vec, fill):
        p = np.full(N_PAD, fill, np.float32)
        p[:n] = vec[lo:hi]
        return np.ascontiguousarray(p.reshape(SLOTS, 128).T)

    mu_t = pad_t(mu, 0.0)
    sig_t = pad_t(sigma, 0.0)
    sigc_t = np.maximum(sig_t, EPS)
    tgt_t = pad_t(target, 1.0)
    ntgtc_t = -np.maximum(tgt_t, EPS)

    return {
        "noise": noise_bt,
        "mu_t": mu_t, "sig_t": sig_t, "sigc_t": sigc_t,
        "tgt_t": tgt_t, "ntgtc_t": ntgtc_t,
        "coef": coef_w,
    }


def _prep_noise(noise):
    """noise [S, N] f32 -> per-core [128, SLOTS*S] bf16 in [p][slot][k]
    layout (column j = lo + slot*128 + p)."""
    import ml_dtypes
    nb = noise.astype(ml_dtypes.bfloat16)
    out = []
    for c in range(NCORES):
        lo = c * N_LOC
        blk = np.zeros((N_PAD, S), ml_dtypes.bfloat16)
        blk[:N_LOC] = nb[:, lo:lo + N_LOC].T
        # [slot, p, k] -> [p, slot, k]
        blk = np.ascontiguousarray(
            blk.reshape(SLOTS, 128, S).transpose(1, 0, 2))
        out.append(blk.reshape(128, SLOTS * S))
    return out


def _run(mu, sigma, target, noise):
    import ml_dtypes
    from concourse import bass_utils

    nc, ntiles, nout = _get_built()

    coef = (2.0 * np.arange(S, dtype=np.float32) + 1.0 - S)  # [S]
    coef_w = np.broadcast_to(np.repeat(coef, 64), (128, S * 64))
    coef_w = np.ascontiguousarray(coef_w).astype(ml_dtypes.bfloat16)

    noise_bt = _prep_noise(noise)
    in_maps = []
    for c in range(NCORES):
        in_maps.append(_prep_core_inputs(
            mu, sigma, target, noise_bt[c],
            c * N_LOC, (c + 1) * N_LOC, coef_w))

    res = bass_utils.run_bass_kernel_spmd(
        nc, in_maps, core_ids=list(range(NCORES)))
    _STATE["last_result"] = res

    t1 = w = iv = 0.0
    for c in range(NCORES):
        p = res.results[c]["partials"].astype(np.float64)
        w += p[:, 0:ntiles].sum()
        t1 += p[:, ntiles].sum()
        iv += p[:, ntiles + 1].sum()
    loss = (t1 / S - w / (S * S) + iv) / N_TOTAL
    return np.float32(loss)


def kernel(mu, sigma, target, noise):
    mu = np.asarray(mu, dtype=np.float32)
    sigma = np.asarray(sigma, dtype=np.float32)
    target = np.asarray(target, dtype=np.float32)
    noise = np.asarray(noise, dtype=np.float32)
    return _run(mu, sigma, target, noise)


# revision 11
# speedup vs baseline: 1.5314x; 1.0817x over previous
"""CombinedCRPSIntervalLoss kernel for 8x TRN2 NeuronCores (v2).

Strategy (pure data parallel over N):
  - shard N across 8 cores; host pre-transposes noise to per-core
    [128 part][slot][k] bf16 so each sort tile DMAs as one contiguous
    line per partition (no PE transpose / PSUM needed).
  - per (tile, column): ACT computes s = exp(mu + sig_c * z) straight
    into a [p, k, c] layout (c contiguous innermost), then |s - t_c|
    with fused accumulation for term1.
  - order statistics via a Batcher merge-exchange network for exactly
    n=100 (Knuth 5.2.2M; 1077 comparators vs 1792 for bitonic-128),
    vectorized with the sort axis strided and columns contiguous so
    every DVE min/max runs in the packed 2x bf16 mode. Ping-pong
    buffer residency is tracked per position at schedule-build time
    (no copy passes); final weighted sum (exact sorted-order CRPS
    identity) reads each residency segment via fused
    tensor_tensor_reduce ops chained through their accumulator.
    The last (short) tile sorts on GpSimd to overlap with DVE.
  - interval score phase is tiny elementwise work on [128, slots].
  - each core emits fp32 partial-sum columns; host combines in fp64.

Math identity used (validated vs reference):
  sum_{i,j}|s_i - s_j| = 2 * sum_k (2k + 1 - S) * s_(k)  (ascending)
"""

import math
import sys

import numpy as np

S = 100
N_TOTAL = 500000
NCORES = 8
N_LOC = N_TOTAL // NCORES          # 62500
SLOTS = 490                        # 490*128 = 62720 >= 62500
N_PAD = SLOTS * 128
TILE_CS = (124, 122, 122, 122)
CMAX = 124
GP_TILES = ()                      # Pool has no ALU tensor ops on TRN2
EPS = 1e-6
ALPHA = 0.1
Z_LO = -1.6448536269514729         # norm.ppf(0.05)
Z_HI = 1.6448536269514722          # norm.ppf(0.95)
PEN_W = 2.0 / ALPHA                # 20.0

_STATE = {}


# ---------------------------------------------------------------------------
# sort schedule: Batcher merge-exchange for n=S with ping-pong residency
# ---------------------------------------------------------------------------

def _merge_exchange_classes(n):
    t = math.ceil(math.log2(n))
    classes = []
    p = 2 ** (t - 1)
    while p >= 1:
        q = 2 ** (t - 1)
        r = 0
        d = p
        while True:
            ilist = [i for i in range(n - d) if (i & p) == r]
            if ilist:
                classes.append((d, ilist))
            if q == p:
                break
            d = q - p
            q //= 2
            r = p
        p //= 2
    return classes


def _decompose_2level(lst):
    """Cover a sorted index list by blocks {a + u*P + v : u<reps, v<bl}."""
    remaining = set(lst)
    blocks = []
    while remaining:
        start = min(remaining)
        bl = 1
        while start + bl in remaining:
            bl += 1
        best = (start, 1, 1, bl)
        best_cov = bl
        for P in (2, 4, 8, 16, 32, 64, 128):
            if P < bl:
                continue
            reps = 1
            while all(start + reps * P + v in remaining for v in range(bl)):
                reps += 1
            cov = reps * bl
            if cov > best_cov:
                best_cov = cov
                best = (start, P, reps, bl)
        blocks.append(best)
        st, P, reps, bl = best
        for u in range(reps):
            for v in range(bl):
                remaining.discard(st + u * P + v)
    return blocks


def _build_sort_schedule(n):
    """Returns (sched, final_segments):
    sched: list of (d, (start, period, reps, blocklen), res_lo, res_hi)
      outputs go to buffers 1-res_lo / 1-res_hi (clobber-free: within a
      class the i and i+d position sets are disjoint since d = p mod 2p).
    final_segments: list of (k0, length, residency) covering [0, n)."""
    res = [0] * n
    sched = []
    for d, il in _merge_exchange_classes(n):
        groups = {}
        for i in il:
            groups.setdefault((res[i], res[i + d]), []).append(i)
        for (rlo, rhi), lst in sorted(groups.items()):
            for blk in _decompose_2level(lst):
                sched.append((d, blk, rlo, rhi))
        for i in il:
            res[i] = 1 - res[i]
            res[i + d] = 1 - res[i + d]
    segs = []
    start = 0
    for k in range(1, n + 1):
        if k == n or res[k] != res[start]:
            segs.append((start, k - start, res[start]))
            start = k
    return sched, segs


_SORT_SCHED, _FINAL_SEGS = _build_sort_schedule(S)


def _selfcheck_schedule():
    rng = np.random.default_rng(0)
    for _ in range(20):
        x = rng.standard_normal(S)
        bufs = [x.copy(), np.zeros(S)]
        for d, (st, P, reps, bl), rlo, rhi in _SORT_SCHED:
            idx = (st + P * np.arange(reps)[:, None]
                   + np.arange(bl)[None, :]).ravel()
            a = bufs[rlo][idx].copy()
            b = bufs[rhi][idx + d].copy()
            bufs[1 - rlo][idx] = np.minimum(a, b)
            bufs[1 - rhi][idx + d] = np.maximum(a, b)
        out = np.empty(S)
        for k0, ln, r in _FINAL_SEGS:
            out[k0:k0 + ln] = bufs[r][k0:k0 + ln]
        assert np.array_equal(out, np.sort(x))


_selfcheck_schedule()


# ---------------------------------------------------------------------------
# axon shim + sync-wait splitting (unchanged from v1)
# ---------------------------------------------------------------------------

def _install_axon_hook_shim():
    """bass_utils imports antenv.axon_hooks when trace=True under axon;
    this image's antenv lacks it. Register a lazy shim so tracing works
    (and trace=False paths are unaffected)."""
    import types
    try:
        import antenv.axon_hooks  # noqa: F401
        return
    except ImportError:
        pass
    mod = types.ModuleType("antenv.axon_hooks")
    _state = {"hook": None, "built": False}

    def set_axon_ntff_profile_hook(h):
        _state["hook"] = h
        _state["built"] = True

    def get_axon_ntff_profile_hook():
        if not _state["built"]:
            _state["built"] = True
            try:
                from trn_agent_boot.trn_boot import _ntff_profile_via_ctypes
                _state["hook"] = _ntff_profile_via_ctypes("/opt/axon/libaxon_pjrt.so")
            except Exception:
                _state["hook"] = None
        return _state["hook"]

    mod.set_axon_ntff_profile_hook = set_axon_ntff_profile_hook
    mod.get_axon_ntff_profile_hook = get_axon_ntff_profile_hook
    sys.modules["antenv.axon_hooks"] = mod
    try:
        import antenv
        antenv.axon_hooks = mod
    except Exception:
        pass


def _split_drain_waits(nc):
    """This walrus build allows only one sem wait per TPB instruction on
    several engine paths (CTRL drain, Pool STT); hoist extra waits onto
    EventSemaphore instructions inserted before (same engine => same
    semantics)."""
    import concourse.mybir as mybir
    for f in nc.m.functions:
        for b in f.blocks:
            new_insts = []
            for inst in b.instructions:
                si = inst.sync_info
                if (not isinstance(inst, mybir.InstEventSemaphore)
                        and si is not None
                        and si.on_wait and len(si.on_wait) > 1):
                    waits = list(si.on_wait)
                    for i, w in enumerate(waits[:-1]):
                        new_insts.append(mybir.InstEventSemaphore(
                            name=f"{inst.name}-dw{i}",
                            engine=inst.engine,
                            ins=[], outs=[],
                            sync_info=mybir.SyncInfo(on_wait=[w], on_update=[]),
                        ))
                    si.on_wait = [waits[-1]]
                new_insts.append(inst)
            b.instructions = new_insts


# ---------------------------------------------------------------------------
# device kernel
# ---------------------------------------------------------------------------

def _mk_ap(bass, buf, koff, P, reps, bl, C):
    """AP over the [128, S, Cdim] tile `buf` selecting k in
    {koff + u*P + v : u<reps, v<bl} x c in [0, C)."""
    v = buf[:, koff:koff + 1, 0:C]
    kstep = v.ap[1][0]
    ap = [v.ap[0]]
    if reps > 1:
        ap.append([P * kstep, reps])
    ap.append([kstep, bl])
    ap.append([1, C])
    return bass.AP(tensor=v.tensor, offset=v.offset, ap=ap)


def _build():
    import concourse.bass as bass
    import concourse.mybir as mybir
    import concourse.tile as tile

    f32 = mybir.dt.float32
    bf16 = mybir.dt.bfloat16
    ntiles = len(TILE_CS)
    nsegs = len(_FINAL_SEGS)
    nout = ntiles * nsegs + 3  # (tile,seg) wsums | t1 | interval halves

    nc = bass.Bass("TRN2", target_bir_lowering=False, debug=False,
                   num_devices=1)

    noise_d = nc.dram_tensor("noise", [128, SLOTS * S], bf16,
                             kind="ExternalInput")
    mu_d = nc.dram_tensor("mu_t", [128, SLOTS], f32, kind="ExternalInput")
    sig_d = nc.dram_tensor("sig_t", [128, SLOTS], f32, kind="ExternalInput")
    sigc_d = nc.dram_tensor("sigc_t", [128, SLOTS], f32, kind="ExternalInput")
    tgt_d = nc.dram_tensor("tgt_t", [128, SLOTS], f32, kind="ExternalInput")
    ntgtc_d = nc.dram_tensor("ntgtc_t", [128, SLOTS], f32,
                             kind="ExternalInput")
    coef_d = nc.dram_tensor("coef", [128, S * CMAX], bf16,
                            kind="ExternalInput")
    part_d = nc.dram_tensor("partials", [128, nout], f32,
                            kind="ExternalOutput")

    aE = mybir.ActivationFunctionType.Exp
    aA = mybir.ActivationFunctionType.Abs
    X = mybir.AxisListType.X
    op_add = mybir.AluOpType.add
    op_sub = mybir.AluOpType.subtract
    op_mul = mybir.AluOpType.mult
    op_min = mybir.AluOpType.min
    op_byp = mybir.AluOpType.bypass
    op_max = mybir.AluOpType.max
    op_lt = mybir.AluOpType.is_lt
    op_gt = mybir.AluOpType.is_gt

    with tile.TileContext(nc) as tc:
        with (
            tc.tile_pool(name="singles", bufs=1) as singles,
            tc.tile_pool(name="nzp", bufs=1) as nzp,
            tc.tile_pool(name="sortp", bufs=2) as sortp,
            tc.tile_pool(name="gpool", bufs=1) as gpool,
        ):
            mu_s = singles.tile([128, SLOTS], f32, tag="mu_s")
            sig_s = singles.tile([128, SLOTS], f32, tag="sig_s")
            sigc_s = singles.tile([128, SLOTS], f32, tag="sigc_s")
            tgt_s = singles.tile([128, SLOTS], f32, tag="tgt_s")
            ntgtc_s = singles.tile([128, SLOTS], f32, tag="ntgtc_s")
            coef_s = singles.tile([128, S * CMAX], bf16, tag="coef_s")
            for sb, dr in ((mu_s, mu_d), (sig_s, sig_d), (sigc_s, sigc_d),
                           (tgt_s, tgt_d), (ntgtc_s, ntgtc_d),
                           (coef_s, coef_d)):
                nc.sync.dma_start(out=sb[:, :], in_=dr.ap())

            t1buf = singles.tile([128, SLOTS], f32, tag="t1buf")
            outbuf = singles.tile([128, nout], f32, tag="outbuf")

            slot_of = []
            s0 = 0
            for C in TILE_CS:
                slot_of.append(s0)
                s0 += C

            def emit_compute(ti, eng_tag):
                """DMA + ACT exp/abs + sort for tile ti on the given engine.
                Returns the (A, B) tile pair for the later weighted sum."""
                C = TILE_CS[ti]
                slot0 = slot_of[ti]
                eng = nc.gpsimd if eng_tag == "g" else nc.vector
                if eng_tag == "g":
                    nz = gpool.tile([128, C * S], bf16, tag=f"nzg{ti}")
                    A = gpool.tile([128, S, C], bf16, tag=f"Ag{ti}")
                    B = gpool.tile([128, S, C], bf16, tag=f"Bg{ti}")
                else:
                    nz = nzp.tile([128, CMAX * S], bf16, tag="nz")
                    A = sortp.tile([128, S, CMAX], bf16, tag="A")
                    B = sortp.tile([128, S, CMAX], bf16, tag="B")
                nc.sync.dma_start(
                    out=nz[:, 0:C * S],
                    in_=noise_d.ap()[:, slot0 * S:(slot0 + C) * S],
                )
                # s = exp(mu + sigc * z), one ACT op per 128-column block
                for c in range(C):
                    f = slot0 + c
                    nc.scalar.activation(
                        A[:, 0:S, c], nz[:, c * S:(c + 1) * S], aE,
                        bias=mu_s[:, f:f + 1], scale=sigc_s[:, f:f + 1],
                    )
                # |s - t_c| with fused column accumulation for term1;
                # write into the (dead) nz region so the out is contiguous
                for c in range(C):
                    f = slot0 + c
                    nc.scalar.activation(
                        nz[:, c * S:(c + 1) * S], A[:, 0:S, c], aA,
                        bias=ntgtc_s[:, f:f + 1],
                        accum_out=t1buf[:, f:f + 1],
                    )
                # merge-exchange sort over k (c contiguous => packed mode).
                # Pool has no TensorTensor opcode; use STT with op0=bypass.
                bufs = (A, B)
                for d, (st, P, reps, bl), rlo, rhi in _SORT_SCHED:
                    a_in = _mk_ap(bass, bufs[rlo], st, P, reps, bl, C)
                    b_in = _mk_ap(bass, bufs[rhi], st + d, P, reps, bl, C)
                    a_out = _mk_ap(bass, bufs[1 - rlo], st, P, reps, bl, C)
                    b_out = _mk_ap(bass, bufs[1 - rhi], st + d, P, reps, bl, C)
                    if eng_tag == "g":
                        eng.scalar_tensor_tensor(
                            out=a_out, in0=a_in, scalar=0.0, in1=b_in,
                            op0=op_byp, op1=op_min)
                        eng.scalar_tensor_tensor(
                            out=b_out, in0=a_in, scalar=0.0, in1=b_in,
                            op0=op_byp, op1=op_max)
                    else:
                        eng.tensor_tensor(out=a_out, in0=a_in, in1=b_in,
                                          op=op_min)
                        eng.tensor_tensor(out=b_out, in0=a_in, in1=b_in,
                                          op=op_max)
                return A, B

            def emit_wsum(ti, A, B):
                """Weighted sum over sorted values (DVE): per-segment
                coef*sorted into the complementary (dead) sort buffer,
                then a fused reduce per segment."""
                C = TILE_CS[ti]
                bufs = (A, B)
                for si, (k0, ln, r) in enumerate(_FINAL_SEGS):
                    in0 = _mk_ap(bass, bufs[r], k0, 1, 1, ln, C)
                    cv = coef_s[:, k0 * CMAX:k0 * CMAX + 1]
                    in1 = bass.AP(tensor=cv.tensor, offset=cv.offset,
                                  ap=[cv.ap[0], [CMAX, ln], [1, C]])
                    out = _mk_ap(bass, bufs[1 - r], k0, 1, 1, ln, C)
                    nc.vector.tensor_tensor(out=out, in0=in0, in1=in1,
                                            op=op_mul)
                    nc.vector.tensor_reduce(
                        out=outbuf[:, ti * nsegs + si:ti * nsegs + si + 1],
                        in_=out, axis=mybir.AxisListType.XY, op=op_add)

            # GpSimd tile first so it starts sorting ASAP
            gp_bufs = {}
            for ti in GP_TILES:
                gp_bufs[ti] = emit_compute(ti, "g")
            dve_tiles = [ti for ti in range(ntiles) if ti not in GP_TILES]
            for ti in dve_tiles:
                A, B = emit_compute(ti, "v")
                emit_wsum(ti, A, B)
            for ti in GP_TILES:
                emit_wsum(ti, *gp_bufs[ti])

            # --- interval score phase, two [128, SLOTS/2] chunks ---
            H = SLOTS // 2
            iv = [singles.tile([128, H], f32, tag=f"iv{i}",
                               name=f"iv{i}") for i in range(7)]
            lo_a, hi_a, low, upp, bel, abv, pen = iv
            for hi_, h0 in enumerate((0, H)):
                sl = slice(h0, h0 + H)
                nc.vector.scalar_tensor_tensor(
                    out=lo_a[:, :], in0=sig_s[:, sl], scalar=Z_LO,
                    in1=mu_s[:, sl], op0=op_mul, op1=op_add)
                nc.vector.scalar_tensor_tensor(
                    out=hi_a[:, :], in0=sig_s[:, sl], scalar=Z_HI,
                    in1=mu_s[:, sl], op0=op_mul, op1=op_add)
                nc.scalar.activation(low[:, :], lo_a[:, :], aE)
                nc.scalar.activation(upp[:, :], hi_a[:, :], aE)
                nc.vector.tensor_tensor(out=bel[:, :], in0=tgt_s[:, sl],
                                        in1=low[:, :], op=op_lt)
                nc.vector.tensor_tensor(out=abv[:, :], in0=tgt_s[:, sl],
                                        in1=upp[:, :], op=op_gt)
                nc.vector.tensor_tensor(out=lo_a[:, :], in0=low[:, :],
                                        in1=tgt_s[:, sl], op=op_sub)
                nc.vector.tensor_tensor(out=hi_a[:, :], in0=tgt_s[:, sl],
                                        in1=upp[:, :], op=op_sub)
                nc.vector.tensor_tensor(out=bel[:, :], in0=lo_a[:, :],
                                        in1=bel[:, :], op=op_mul)
                nc.vector.tensor_tensor(out=abv[:, :], in0=hi_a[:, :],
                                        in1=abv[:, :], op=op_mul)
                nc.vector.tensor_tensor(out=pen[:, :], in0=bel[:, :],
                                        in1=abv[:, :], op=op_add)
                nc.vector.tensor_tensor(out=upp[:, :], in0=upp[:, :],
                                        in1=low[:, :], op=op_sub)
                nc.vector.scalar_tensor_tensor(
                    out=low[:, :], in0=pen[:, :], scalar=PEN_W,
                    in1=upp[:, :], op0=op_mul, op1=op_add,
                    accum_out=outbuf[:, nout - 2 + hi_:nout - 1 + hi_])

            # term1 total
            nc.vector.tensor_reduce(
                out=outbuf[:, nout - 3:nout - 2], in_=t1buf[:, :],
                axis=X, op=op_add)
            nc.sync.dma_start(out=part_d.ap(), in_=outbuf[:, :])

    _split_drain_waits(nc)
    return nc, ntiles, nout


def _get_built():
    if "nc" not in _STATE:
        _install_axon_hook_shim()
        _STATE["nc"] = _build()
    return _STATE["nc"]


# ---------------------------------------------------------------------------
# host-side data marshaling
# ---------------------------------------------------------------------------

def _prep_core_inputs(mu, sigma, target, noise_bt, lo, hi, coef_w):
    n = hi - lo

    def pad_t(vec, fill):
        p = np.full(N_PAD, fill, np.float32)
        p[:n] = vec[lo:hi]
        return np.ascontiguousarray(p.reshape(SLOTS, 128).T)

    mu_t = pad_t(mu, 0.0)
    sig_t = pad_t(sigma, 0.0)
    sigc_t = np.maximum(sig_t, EPS)
    tgt_t = pad_t(target, 1.0)
    ntgtc_t = -np.maximum(tgt_t, EPS)

    return {
        "noise": noise_bt,
        "mu_t": mu_t, "sig_t": sig_t, "sigc_t": sigc_t,
        "tgt_t": tgt_t, "ntgtc_t": ntgtc_t,
        "coef": coef_w,
    }


def _prep_noise(noise):
    """noise [S, N] f32 -> per-core [128, SLOTS*S] bf16 in [p][slot][k]
    layout (column j = lo + slot*128 + p)."""
    import ml_dtypes
    nb = noise.astype(ml_dtypes.bfloat16)
    out = []
    for c in range(NCORES):
        lo = c * N_LOC
        blk = np.zeros((N_PAD, S), ml_dtypes.bfloat16)
        blk[:N_LOC] = nb[:, lo:lo + N_LOC].T
        # [slot, p, k] -> [p, slot, k]
        blk = np.ascontiguousarray(
            blk.reshape(SLOTS, 128, S).transpose(1, 0, 2))
        out.append(blk.reshape(128, SLOTS * S))
    return out


def _run(mu, sigma, target, noise):
    import ml_dtypes
    from concourse import bass_utils

    nc, ntiles, nout = _get_built()

    coef = (2.0 * np.arange(S, dtype=np.float32) + 1.0 - S)  # [S]
    coef_w = np.broadcast_to(np.repeat(coef, CMAX), (128, S * CMAX))
    coef_w = np.ascontiguousarray(coef_w).astype(ml_dtypes.bfloat16)

    noise_bt = _prep_noise(noise)
    in_maps = []
    for c in range(NCORES):
        in_maps.append(_prep_core_inputs(
            mu, sigma, target, noise_bt[c],
            c * N_LOC, (c + 1) * N_LOC, coef_w))

    res = bass_utils.run_bass_kernel_spmd(
        nc, in_maps, core_ids=list(range(NCORES)))
    _STATE["last_result"] = res

    t1 = w = iv = 0.0
    for c in range(NCORES):
        p = res.results[c]["partials"].astype(np.float64)
        w += p[:, 0:nout - 3].sum()
        t1 += p[:, nout - 3].sum()
        iv += p[:, nout - 2:nout].sum()
    loss = (t1 / S - w / (S * S) + iv) / N_TOTAL
    return np.float32(loss)


def kernel(mu, sigma, target, noise):
    mu = np.asarray(mu, dtype=np.float32)
    sigma = np.asarray(sigma, dtype=np.float32)
    target = np.asarray(target, dtype=np.float32)
    noise = np.asarray(noise, dtype=np.float32)
    return _run(mu, sigma, target, noise)
